# revision 11
# baseline (speedup 1.0000x reference)
"""AngProtoLoss (stable) distributed Bass kernel for 8 TRN2 NeuronCores.

Problem (reference):
    dvecs: (4096, 16, 512) f32
    centroids = mean(dvecs, axis=1)                  # (N, D)
    u = dvecs[:, -1, :]                              # (N, D)
    cos = clip(cos_sim(u, centroids), min=1e-6)      # (N, N)
    logits = cos * w + b
    loss = -mean(diag(log_softmax(logits)))
        = mean_i [ logsumexp_k(w*clip(cos_ik)) - w*clip(cos_ii) ]   (b cancels)

Sharding: data-parallel over speakers N. Each core gets 512 speakers,
computes local (normalized) centroids, all-gathers them (bf16), computes its
512 rows of the cos matrix, the local log-sum-exp terms and local diagonal
terms, and outputs 512 per-row loss terms. Host sums and divides by N.
"""

import os
import sys

for _p in ("/opt/trn_rl_repo",):
    if os.path.isdir(_p) and _p not in sys.path:
        sys.path.append(_p)

import numpy as np

import concourse.bass as bass
import concourse.tile as tile
from concourse import bacc, mybir
from concourse.bass_utils import run_bass_kernel_spmd
from concourse.masks import make_identity

N_CORES = 8
N, M, D = 4096, 16, 512
P = 128                     # partitions
LOCAL = N // N_CORES        # 512 speakers per core
NCHUNK = LOCAL // P         # 4 chunks of 128 speakers
NT = D // P                 # 4 d-tiles
EPS = 1e-6

F32 = mybir.dt.float32
BF16 = mybir.dt.bfloat16
AF = mybir.ActivationFunctionType


def build_program(w_val: float):
    nc = bacc.Bacc("TRN2", target_bir_lowering=False, debug=False,
                   num_devices=N_CORES)
    dvecs = nc.dram_tensor("dvecs", [LOCAL, M, D], F32, kind="ExternalInput").ap()
    out = nc.dram_tensor("out", [LOCAL], F32, kind="ExternalOutput").ap()

    with tile.TileContext(nc) as tc:
        _build(nc, tc, dvecs, out, w_val)
    nc.compile()
    return nc


def _build(nc, tc, dvecs, out, w_val):
    from contextlib import ExitStack
    ctx = ExitStack()
    with ctx:
        singles = ctx.enter_context(tc.tile_pool(name="singles", bufs=1))
        xpool = ctx.enter_context(tc.tile_pool(name="xpool", bufs=2))
        tree = ctx.enter_context(tc.tile_pool(name="tree", bufs=2))
        cpool = ctx.enter_context(tc.tile_pool(name="cpool", bufs=2))
        stats = ctx.enter_context(tc.tile_pool(name="stats", bufs=4))
        gpool = ctx.enter_context(tc.tile_pool(name="gpool", bufs=NCHUNK))
        epool = ctx.enter_context(tc.tile_pool(name="epool", bufs=3))
        tpsum = ctx.enter_context(tc.tile_pool(name="tpsum", bufs=2, space="PSUM"))
        mpsum = ctx.enter_context(tc.tile_pool(name="mpsum", bufs=2, space="PSUM"))
        dram = ctx.enter_context(tc.tile_pool(name="dram", bufs=1, space="DRAM"))

        ident = singles.tile([P, P], BF16)
        make_identity(nc, ident)

        # Tiny dummy collective issued first: absorbs the one-time
        # cc-stream init (~50us) under the load phase.
        dummy_in = dram.tile([1, 128], BF16, name="dummy_in")
        dummy_out = dram.tile([N_CORES, 128], BF16, name="dummy_out",
                              addr_space="Shared")
        dummy_sb = singles.tile([1, 128], BF16)
        nc.vector.memset(dummy_sb, 0.0)
        nc.sync.dma_start(out=dummy_in, in_=dummy_sb)
        nc.gpsimd.collective_compute(
            "AllGather", mybir.AluOpType.bypass,
            replica_groups=[list(range(N_CORES))],
            ins=[dummy_in.opt()], outs=[dummy_out.opt()],
        )

        # persistent across the whole kernel
        uT = singles.tile([P, NT, LOCAL], BF16)          # u^T: [d_in_tile, t, i]
        s_acc = singles.tile([P, NCHUNK], F32)           # sum_k exp(w*clip(cos)), per q
        diag_all = singles.tile([P, NCHUNK], F32)        # diag cos, per q
        rows = singles.tile([P, NCHUNK], F32)            # per-row loss terms
        nc.vector.memset(s_acc, 0.0)

        gath = []   # per-chunk allgather outputs (DRAM, Shared)
        g_sbs = []  # per-chunk gathered-centroid SBUF tiles
        for r in range(NCHUNK):
            # ---- load chunk r: [128, 16, 512] f32 -> bf16 (cast in DMA) ----
            x = xpool.tile([P, M, D], BF16, name=f"x{r}", tag="x")
            nc.gpsimd.dma_start(out=x, in_=dvecs[r * P:(r + 1) * P, :, :])

            # ---- centroid sum over m (tree) ----
            t1 = tree.tile([P, M // 2, D], BF16, name=f"t1_{r}", tag="t1")
            for j in range(M // 2):
                nc.vector.tensor_add(t1[:, j, :], x[:, 2 * j, :], x[:, 2 * j + 1, :])
            t2 = tree.tile([P, M // 4, D], BF16, name=f"t2_{r}", tag="t2")
            for j in range(M // 4):
                nc.vector.tensor_add(t2[:, j, :], t1[:, 2 * j, :], t1[:, 2 * j + 1, :])
            t3 = tree.tile([P, M // 8, D], BF16, name=f"t3_{r}", tag="t3")
            for j in range(M // 8):
                nc.vector.tensor_add(t3[:, j, :], t2[:, 2 * j, :], t2[:, 2 * j + 1, :])
            csum = cpool.tile([P, D], BF16, name=f"csum{r}", tag="csum")
            nc.vector.tensor_add(csum, t3[:, 0, :], t3[:, 1, :])

            u = x[:, M - 1, :]   # last utterance (bf16 view)

            # ---- norms: scale = rsqrt(sum(v^2)) = exp(-0.5*ln(ssq)) ----
            sq_scr = cpool.tile([P, D], BF16, name=f"sqscr{r}", tag="sqscr")
            ssq_c = stats.tile([P, 1], F32, name=f"ssqc{r}", tag="ssqc")
            ssq_u = stats.tile([P, 1], F32, name=f"ssqu{r}", tag="ssqu")
            nc.vector.tensor_mul(sq_scr, csum, csum)
            nc.vector.tensor_reduce(ssq_c, sq_scr,
                                    axis=mybir.AxisListType.X,
                                    op=mybir.AluOpType.add)
            nc.vector.tensor_mul(sq_scr, u, u)
            nc.vector.tensor_reduce(ssq_u, sq_scr,
                                    axis=mybir.AxisListType.X,
                                    op=mybir.AluOpType.add)
            lc = stats.tile([P, 1], F32, name=f"lc{r}", tag="lc")
            lu = stats.tile([P, 1], F32, name=f"lu{r}", tag="lu")
            nc.scalar.activation(lc, ssq_c, AF.Ln)
            nc.scalar.activation(lu, ssq_u, AF.Ln)
            rc = stats.tile([P, 1], F32, name=f"rc{r}", tag="rc")
            ru = stats.tile([P, 1], F32, name=f"ru{r}", tag="ru")
            nc.scalar.activation(rc, lc, AF.Exp, scale=-0.5)
            nc.scalar.activation(ru, lu, AF.Exp, scale=-0.5)

            # ---- normalize ----
            chat = cpool.tile([P, D], BF16, name=f"chat{r}", tag="chat")
            uhat = cpool.tile([P, D], BF16, name=f"uhat{r}", tag="uhat")
            nc.vector.tensor_scalar_mul(chat, csum, rc)
            nc.vector.tensor_scalar_mul(uhat, u, ru)

            # ---- diagonal cos (local) ----
            dg_scr = cpool.tile([P, D], BF16, name=f"dgscr{r}", tag="dgscr")
            nc.vector.tensor_mul(dg_scr, chat, uhat)
            nc.vector.tensor_reduce(
                diag_all[:, r:r + 1], dg_scr,
                axis=mybir.AxisListType.X, op=mybir.AluOpType.add,
            )

            # ---- transposes (PE): chat/uhat [128i, 512d] -> [128d, i] tiles ----
            cT = cpool.tile([P, NT, P], BF16, name=f"cT{r}", tag="cT")
            for t in range(NT):
                pt = tpsum.tile([P, P], BF16, name=f"ptc{r}_{t}", tag="ptc")
                nc.tensor.transpose(pt, chat[:, t * P:(t + 1) * P], ident)
                nc.vector.tensor_copy(cT[:, t, :], pt)
                pu = tpsum.tile([P, P], BF16, name=f"ptu{r}_{t}", tag="ptu")
                nc.tensor.transpose(pu, uhat[:, t * P:(t + 1) * P], ident)
                nc.vector.tensor_copy(uT[:, t, r * P:(r + 1) * P], pu)

            # ---- allgather chunk r of normalized centroids (bf16) ----
            bounce = dram.tile([D, P], BF16, name=f"bounce{r}")
            nc.sync.dma_start(out=bounce.rearrange("(t p) i -> p t i", p=P),
                              in_=cT)
            g = dram.tile([N_CORES * D, P], BF16, name=f"gath{r}",
                          addr_space="Shared")
            nc.gpsimd.collective_compute(
                "AllGather", mybir.AluOpType.bypass,
                replica_groups=[list(range(N_CORES))],
                ins=[bounce.opt()], outs=[g.opt()],
            )
            gath.append(g)

            # ---- bring gathered block to SBUF: [128, (c,t), 128] ----
            g_sb = gpool.tile([P, N_CORES * NT, P], BF16, name=f"gsb{r}", tag="gsb")
            nc.sync.dma_start(out=g_sb,
                              in_=g.rearrange("(c t p) i -> p (c t) i", t=NT, p=P))

            g_sbs.append(g_sb)

            # ---- matmuls + epilogue for every (row-chunk q, gathered chunk)
            # pair that just became available: rows q<=r vs this gather, plus
            # row r vs all earlier gathers.
            for q, rr in [(qq, r) for qq in range(r + 1)] + \
                         [(r, rr) for rr in range(r)]:
                gsb = g_sbs[rr]
                ps = mpsum.tile([P, 2, N_CORES // 2 * P], F32,
                                name=f"ps{rr}_{q}", tag="ps")
                for h in range(2):
                    for t in range(NT):
                        # rhs: ranks c in [4h, 4h+4), d-tile t -> N=512
                        rhs = gsb[:, 16 * h + t:16 * h + t + 13:NT, :]
                        nc.tensor.matmul(
                            ps[:, h, :],
                            uT[:, t, q * P:(q + 1) * P],
                            rhs,
                            start=(t == 0), stop=(t == NT - 1),
                        )
                # epilogue: y = max(cos, eps); s += sum_k exp(w*y)
                y = epool.tile([P, 2 * (N_CORES // 2) * P], BF16,
                               name=f"y{rr}_{q}", tag="y")
                nc.vector.tensor_scalar_max(y, ps.rearrange("p a b -> p (a b)"), EPS)
                e_scr = epool.tile([P, 2 * (N_CORES // 2) * P], BF16,
                                   name=f"escr{rr}_{q}", tag="escr")
                s_part = stats.tile([P, 1], F32, name=f"sp{rr}_{q}", tag="sp")
                nc.scalar.activation(e_scr, y, AF.Exp, scale=w_val,
                                     accum_out=s_part)
                nc.vector.tensor_add(s_acc[:, q:q + 1], s_acc[:, q:q + 1], s_part)

        # ---- finals: rows = ln(s) - w*max(diag, eps) ----
        dgc = singles.tile([P, NCHUNK], F32)
        nc.vector.tensor_scalar_max(dgc, diag_all, EPS)
        lse = singles.tile([P, NCHUNK], F32)
        nc.scalar.activation(lse, s_acc, AF.Ln)
        nc.vector.tensor_scalar(
            rows, dgc, -w_val, None, mybir.AluOpType.mult)
        nc.vector.tensor_add(rows, rows, lse)
        nc.sync.dma_start(out=out.rearrange("(q p) -> p q", p=P), in_=rows)


_CACHE = {}


def kernel(dvecs, w, b):
    w_val = float(np.asarray(w))
    key = w_val
    if key not in _CACHE:
        _CACHE[key] = build_program(w_val)
    nc = _CACHE[key]
    dvecs = np.ascontiguousarray(np.asarray(dvecs, dtype=np.float32))
    in_maps = [
        {"dvecs": dvecs[c * LOCAL:(c + 1) * LOCAL]} for c in range(N_CORES)
    ]
    res = run_bass_kernel_spmd(nc, in_maps, core_ids=list(range(N_CORES)))
    total = 0.0
    for c in range(N_CORES):
        total += float(np.asarray(res.results[c]["out"], dtype=np.float64).sum())
    return np.float32(total / N)


# revision 12
# speedup vs baseline: 1.2246x; 1.2246x over previous
"""AngProtoLoss (stable) distributed Bass kernel for 8 TRN2 NeuronCores.

Problem (reference):
    dvecs: (4096, 16, 512) f32
    centroids = mean(dvecs, axis=1)                  # (N, D)
    u = dvecs[:, -1, :]                              # (N, D)
    cos = clip(cos_sim(u, centroids), min=1e-6)      # (N, N)
    logits = cos * w + b
    loss = -mean(diag(log_softmax(logits)))
        = mean_i [ logsumexp_k(w*clip(cos_ik)) - w*clip(cos_ii) ]   (b cancels)

Sharding: data-parallel over speakers N. Each core gets 512 speakers,
computes local (normalized) centroids, all-gathers them (bf16), computes its
512 rows of the cos matrix, the local log-sum-exp terms and local diagonal
terms, and outputs 512 per-row loss terms. Host sums and divides by N.

Schedule notes (engine queues are FIFO):
 - gpsimd queue holds ONLY the collectives (plus identity init) so each
   allgather triggers as soon as its bounce buffer is written; a tiny dummy
   allgather goes first to absorb the ~50us one-time cc-stream init under
   the load phase.
 - all DMAs ride the sync HWDGE ring in data-ready order:
   loads, bounce writes, gathered reads, output.
 - all PE transposes are emitted before any big matmul so the PE queue never
   stalls a chunk's transpose behind a matmul waiting on a gather.
"""

import os
import sys

for _p in ("/opt/trn_rl_repo",):
    if os.path.isdir(_p) and _p not in sys.path:
        sys.path.append(_p)

import numpy as np

import concourse.bass as bass
import concourse.tile as tile
from concourse import bacc, mybir
from concourse.bass_utils import run_bass_kernel_spmd
from concourse.masks import make_identity

N_CORES = 8
N, M, D = 4096, 16, 512
P = 128                     # partitions
LOCAL = N // N_CORES        # 512 speakers per core
NCHUNK = LOCAL // P         # 4 chunks of 128 speakers
NT = D // P                 # 4 d-tiles
EPS = 1e-6

F32 = mybir.dt.float32
BF16 = mybir.dt.bfloat16
AF = mybir.ActivationFunctionType


def build_program(w_val: float):
    nc = bacc.Bacc("TRN2", target_bir_lowering=False, debug=False,
                   num_devices=N_CORES)
    dvecs = nc.dram_tensor("dvecs", [LOCAL, M, D], F32, kind="ExternalInput").ap()
    out = nc.dram_tensor("out", [LOCAL], F32, kind="ExternalOutput").ap()

    with tile.TileContext(nc) as tc:
        _build(nc, tc, dvecs, out, w_val)
    nc.compile()
    return nc


def _build(nc, tc, dvecs, out, w_val):
    from contextlib import ExitStack
    ctx = ExitStack()
    with ctx:
        singles = ctx.enter_context(tc.tile_pool(name="singles", bufs=1))
        xpool = ctx.enter_context(tc.tile_pool(name="xpool", bufs=2))
        tree = ctx.enter_context(tc.tile_pool(name="tree", bufs=2))
        cpool = ctx.enter_context(tc.tile_pool(name="cpool", bufs=2))
        stats = ctx.enter_context(tc.tile_pool(name="stats", bufs=4))
        gpool = ctx.enter_context(tc.tile_pool(name="gpool", bufs=NCHUNK))
        epool = ctx.enter_context(tc.tile_pool(name="epool", bufs=3))
        tpsum = ctx.enter_context(tc.tile_pool(name="tpsum", bufs=2, space="PSUM"))
        mpsum = ctx.enter_context(tc.tile_pool(name="mpsum", bufs=2, space="PSUM"))
        dram = ctx.enter_context(tc.tile_pool(name="dram", bufs=1, space="DRAM"))

        # Tiny dummy collective first: absorbs the one-time cc-stream init.
        with tc.high_priority():
            dummy_in = dram.tile([1, 16], BF16, name="dummy_in")
            dummy_out = dram.tile([N_CORES, 16], BF16, name="dummy_out",
                                  addr_space="Shared")
            dummy_sb = singles.tile([1, 16], BF16)
            nc.vector.memset(dummy_sb, 0.0)
            nc.sync.dma_start(out=dummy_in, in_=dummy_sb)
            nc.gpsimd.collective_compute(
                "AllGather", mybir.AluOpType.bypass,
                replica_groups=[list(range(N_CORES))],
                ins=[dummy_in.opt()], outs=[dummy_out.opt()],
            )

        ident = singles.tile([P, P], F32)
        make_identity(nc, ident)

        # persistent across the whole kernel
        uT = singles.tile([P, NT, LOCAL], BF16)          # u^T: [d_in_tile, t, i]
        s_acc = singles.tile([P, NCHUNK], F32)           # sum_k exp(w*clip(cos))
        diag_all = singles.tile([P, NCHUNK], F32)        # diag cos, per q
        rows = singles.tile([P, NCHUNK], F32)            # per-row loss terms
        nc.vector.memset(s_acc, 0.0)

        # ---------- phase A: loads first (sync ring order) ----------
        xs = []
        for r in range(NCHUNK):
            x = xpool.tile([P, M, D], F32, name=f"x{r}", tag="x")
            nc.sync.dma_start(out=x, in_=dvecs[r * P:(r + 1) * P, :, :])
            xs.append(x)

        # ---------- phase B: per-chunk centroid pipeline + allgather ----------
        gath = []
        for r in range(NCHUNK):
            x = xs[r]
            # centroid sum over m: first level casts f32 -> bf16
            t1 = tree.tile([P, M // 2, D], BF16, name=f"t1_{r}", tag="t1")
            for j in range(M // 2):
                nc.vector.tensor_add(t1[:, j, :], x[:, 2 * j, :], x[:, 2 * j + 1, :])
            t2 = tree.tile([P, M // 4, D], BF16, name=f"t2_{r}", tag="t2")
            for j in range(M // 4):
                nc.vector.tensor_add(t2[:, j, :], t1[:, 2 * j, :], t1[:, 2 * j + 1, :])
            t3 = tree.tile([P, M // 8, D], BF16, name=f"t3_{r}", tag="t3")
            for j in range(M // 8):
                nc.vector.tensor_add(t3[:, j, :], t2[:, 2 * j, :], t2[:, 2 * j + 1, :])
            csum = cpool.tile([P, D], BF16, name=f"csum{r}", tag="csum")
            nc.vector.tensor_add(csum, t3[:, 0, :], t3[:, 1, :])

            u = x[:, M - 1, :]   # last utterance (f32 view)

            # norms: scale = rsqrt(ssq) = exp(-0.5*ln(ssq))
            sq_scr = cpool.tile([P, D], BF16, name=f"sqscr{r}", tag="sqscr")
            ssq_c = stats.tile([P, 1], F32, name=f"ssqc{r}", tag="ssqc")
            ssq_u = stats.tile([P, 1], F32, name=f"ssqu{r}", tag="ssqu")
            nc.vector.tensor_mul(sq_scr, csum, csum)
            nc.vector.tensor_reduce(ssq_c, sq_scr, axis=mybir.AxisListType.X,
                                    op=mybir.AluOpType.add)
            nc.vector.tensor_mul(sq_scr, u, u)
            nc.vector.tensor_reduce(ssq_u, sq_scr, axis=mybir.AxisListType.X,
                                    op=mybir.AluOpType.add)
            lc = stats.tile([P, 1], F32, name=f"lc{r}", tag="lc")
            lu = stats.tile([P, 1], F32, name=f"lu{r}", tag="lu")
            nc.scalar.activation(lc, ssq_c, AF.Ln)
            nc.scalar.activation(lu, ssq_u, AF.Ln)
            rc = stats.tile([P, 1], F32, name=f"rc{r}", tag="rc")
            ru = stats.tile([P, 1], F32, name=f"ru{r}", tag="ru")
            nc.scalar.activation(rc, lc, AF.Exp, scale=-0.5)
            nc.scalar.activation(ru, lu, AF.Exp, scale=-0.5)

            # normalize (f32 out so the PSUM->SBUF copies can ride ScalarE)
            chat = cpool.tile([P, D], F32, name=f"chat{r}", tag="chat")
            uhat = cpool.tile([P, D], F32, name=f"uhat{r}", tag="uhat")
            nc.vector.tensor_scalar_mul(chat, csum, rc)
            nc.vector.tensor_scalar_mul(uhat, u, ru)

            # diagonal cos (local)
            dg_scr = cpool.tile([P, D], F32, name=f"dgscr{r}", tag="dgscr")
            nc.vector.tensor_mul(dg_scr, chat, uhat)
            nc.vector.tensor_reduce(diag_all[:, r:r + 1], dg_scr,
                                    axis=mybir.AxisListType.X,
                                    op=mybir.AluOpType.add)

            # transposes on PE (f32 in -> f32 psum), cast to bf16 on ScalarE
            cT = cpool.tile([P, NT, P], BF16, name=f"cT{r}", tag="cT")
            for t in range(NT):
                pt = tpsum.tile([P, P], F32, name=f"ptc{r}_{t}", tag="ptc")
                nc.tensor.transpose(pt, chat[:, t * P:(t + 1) * P], ident)
                nc.scalar.copy(cT[:, t, :], pt)
                pu = tpsum.tile([P, P], F32, name=f"ptu{r}_{t}", tag="ptu")
                nc.tensor.transpose(pu, uhat[:, t * P:(t + 1) * P], ident)
                nc.scalar.copy(uT[:, t, r * P:(r + 1) * P], pu)

            # bounce write + allgather (bf16)
            bounce = dram.tile([D, P], BF16, name=f"bounce{r}")
            nc.sync.dma_start(out=bounce.rearrange("(t p) i -> p t i", p=P),
                              in_=cT)
            g = dram.tile([N_CORES * D, P], BF16, name=f"gath{r}",
                          addr_space="Shared")
            nc.gpsimd.collective_compute(
                "AllGather", mybir.AluOpType.bypass,
                replica_groups=[list(range(N_CORES))],
                ins=[bounce.opt()], outs=[g.opt()],
            )
            gath.append(g)

        # ---------- phase C: gathered reads + matmuls + epilogue ----------
        for r in range(NCHUNK):
            g_sb = gpool.tile([P, N_CORES * NT, P], BF16, name=f"gsb{r}",
                              tag="gsb")
            nc.sync.dma_start(
                out=g_sb,
                in_=gath[r].rearrange("(c t p) i -> p (c t) i", t=NT, p=P))
            for q in range(NCHUNK):
                ps = mpsum.tile([P, 2, N_CORES // 2 * P], F32,
                                name=f"ps{r}_{q}", tag="ps")
                for h in range(2):
                    for t in range(NT):
                        # rhs: ranks c in [4h, 4h+4), d-tile t -> N=512
                        rhs = g_sb[:, 16 * h + t:16 * h + t + 13:NT, :]
                        nc.tensor.matmul(
                            ps[:, h, :],
                            uT[:, t, q * P:(q + 1) * P],
                            rhs,
                            start=(t == 0), stop=(t == NT - 1),
                        )
                # epilogue: y = max(cos, eps); s += sum_k exp(w*y)
                y = epool.tile([P, 2 * (N_CORES // 2) * P], BF16,
                               name=f"y{r}_{q}", tag="y")
                nc.vector.tensor_scalar_max(y, ps.rearrange("p a b -> p (a b)"),
                                            EPS)
                e_scr = epool.tile([P, 2 * (N_CORES // 2) * P], BF16,
                                   name=f"escr{r}_{q}", tag="escr")
                s_part = stats.tile([P, 1], F32, name=f"sp{r}_{q}", tag="sp")
                nc.scalar.activation(e_scr, y, AF.Exp, scale=w_val,
                                     accum_out=s_part)
                nc.vector.tensor_add(s_acc[:, q:q + 1], s_acc[:, q:q + 1],
                                     s_part)

        # ---------- finals: rows = ln(s) - w*max(diag, eps) ----------
        dgc = singles.tile([P, NCHUNK], F32)
        nc.vector.tensor_scalar_max(dgc, diag_all, EPS)
        lse = singles.tile([P, NCHUNK], F32)
        nc.scalar.activation(lse, s_acc, AF.Ln)
        nc.vector.tensor_scalar(rows, dgc, -w_val, None, mybir.AluOpType.mult)
        nc.vector.tensor_add(rows, rows, lse)
        nc.sync.dma_start(out=out.rearrange("(q p) -> p q", p=P), in_=rows)


_CACHE = {}


def kernel(dvecs, w, b):
    w_val = float(np.asarray(w))
    key = w_val
    if key not in _CACHE:
        _CACHE[key] = build_program(w_val)
    nc = _CACHE[key]
    dvecs = np.ascontiguousarray(np.asarray(dvecs, dtype=np.float32))
    in_maps = [
        {"dvecs": dvecs[c * LOCAL:(c + 1) * LOCAL]} for c in range(N_CORES)
    ]
    res = run_bass_kernel_spmd(nc, in_maps, core_ids=list(range(N_CORES)))
    total = 0.0
    for c in range(N_CORES):
        total += float(np.asarray(res.results[c]["out"], dtype=np.float64).sum())
    return np.float32(total / N)


# revision 15
# speedup vs baseline: 1.5527x; 1.2679x over previous
"""AngProtoLoss (stable) distributed Bass kernel for 8 TRN2 NeuronCores.

Problem (reference):
    dvecs: (4096, 16, 512) f32
    centroids = mean(dvecs, axis=1)                  # (N, D)
    u = dvecs[:, -1, :]                              # (N, D)
    cos = clip(cos_sim(u, centroids), min=1e-6)      # (N, N)
    logits = cos * w + b
    loss = -mean(diag(log_softmax(logits)))
        = mean_i [ logsumexp_k(w*clip(cos_ik)) - w*clip(cos_ii) ]   (b cancels)

Sharding: data-parallel over speakers N. Each core gets 512 speakers,
computes local (normalized) centroids, all-gathers them (bf16), computes its
512 rows of the cos matrix, the local log-sum-exp terms and local diagonal
terms, and outputs 512 per-row loss terms. Host sums and divides by N.

Schedule notes (engine queues are FIFO):
 - gpsimd queue holds ONLY the collectives (plus identity init) so each
   allgather triggers as soon as its bounce buffer is written; a tiny dummy
   allgather goes first to absorb the ~50us one-time cc-stream init under
   the load phase.
 - all DMAs ride the sync HWDGE ring in data-ready order:
   loads, bounce writes, gathered reads, output.
 - all PE transposes are emitted before any big matmul so the PE queue never
   stalls a chunk's transpose behind a matmul waiting on a gather.
"""

import os
import sys

for _p in ("/opt/trn_rl_repo",):
    if os.path.isdir(_p) and _p not in sys.path:
        sys.path.append(_p)

import numpy as np

import concourse.bass as bass
import concourse.tile as tile
from concourse import bacc, mybir
from concourse.bass_utils import run_bass_kernel_spmd
from concourse.masks import make_identity

N_CORES = 8
N, M, D = 4096, 16, 512
P = 128                     # partitions
LOCAL = N // N_CORES        # 512 speakers per core
NCHUNK = LOCAL // P         # 4 chunks of 128 speakers
NT = D // P                 # 4 d-tiles
EPS = 1e-6

F32 = mybir.dt.float32
BF16 = mybir.dt.bfloat16
FP8 = mybir.dt.float8e4
AF = mybir.ActivationFunctionType


def build_program(w_val: float):
    nc = bacc.Bacc("TRN2", target_bir_lowering=False, debug=False,
                   num_devices=N_CORES)
    dvecs = nc.dram_tensor("dvecs", [LOCAL, M, D], F32, kind="ExternalInput").ap()
    out = nc.dram_tensor("out", [LOCAL], F32, kind="ExternalOutput").ap()

    with tile.TileContext(nc) as tc:
        _build(nc, tc, dvecs, out, w_val)
    nc.compile()
    return nc


def _build(nc, tc, dvecs, out, w_val):
    from contextlib import ExitStack
    ctx = ExitStack()
    with ctx:
        singles = ctx.enter_context(tc.tile_pool(name="singles", bufs=1))
        xpool = ctx.enter_context(tc.tile_pool(name="xpool", bufs=2))
        tree = ctx.enter_context(tc.tile_pool(name="tree", bufs=2))
        cpool = ctx.enter_context(tc.tile_pool(name="cpool", bufs=2))
        stats = ctx.enter_context(tc.tile_pool(name="stats", bufs=4))
        gpool = ctx.enter_context(tc.tile_pool(name="gpool", bufs=NCHUNK))
        epool = ctx.enter_context(tc.tile_pool(name="epool", bufs=3))
        tpsum = ctx.enter_context(tc.tile_pool(name="tpsum", bufs=2, space="PSUM"))
        mpsum = ctx.enter_context(tc.tile_pool(name="mpsum", bufs=2, space="PSUM"))
        dram = ctx.enter_context(tc.tile_pool(name="dram", bufs=1, space="DRAM"))

        ident = singles.tile([P, P], F32)
        make_identity(nc, ident)

        # persistent across the whole kernel
        uT = singles.tile([P, NT, LOCAL], BF16)          # u^T: [d_in_tile, t, i]
        s_acc = singles.tile([P, NCHUNK], F32)           # sum_k exp(w*clip(cos))
        diag_all = singles.tile([P, NCHUNK], F32)        # diag cos, per q
        rows = singles.tile([P, NCHUNK], F32)            # per-row loss terms
        nc.vector.memset(s_acc, 0.0)

        # ---------- phase A: loads first (sync ring order) ----------
        xs = []
        for r in range(NCHUNK):
            x = xpool.tile([P, M, D], F32, name=f"x{r}", tag="x")
            nc.sync.dma_start(out=x, in_=dvecs[r * P:(r + 1) * P, :, :])
            xs.append(x)

        # ---------- phase B: per-chunk centroid pipeline + allgather ----------
        gath = []
        for r in range(NCHUNK):
            x = xs[r]
            # centroid sum over m: first level casts f32 -> bf16
            t1 = tree.tile([P, M // 2, D], BF16, name=f"t1_{r}", tag="t1")
            for j in range(M // 2):
                nc.vector.tensor_add(t1[:, j, :], x[:, 2 * j, :], x[:, 2 * j + 1, :])
            t2 = tree.tile([P, M // 4, D], BF16, name=f"t2_{r}", tag="t2")
            for j in range(M // 4):
                nc.vector.tensor_add(t2[:, j, :], t1[:, 2 * j, :], t1[:, 2 * j + 1, :])
            t3 = tree.tile([P, M // 8, D], BF16, name=f"t3_{r}", tag="t3")
            for j in range(M // 8):
                nc.vector.tensor_add(t3[:, j, :], t2[:, 2 * j, :], t2[:, 2 * j + 1, :])
            csum = cpool.tile([P, D], BF16, name=f"csum{r}", tag="csum")
            nc.vector.tensor_add(csum, t3[:, 0, :], t3[:, 1, :])

            u = x[:, M - 1, :]   # last utterance (f32 view)

            # norms: scale = rsqrt(ssq) = exp(-0.5*ln(ssq))
            sq_scr = cpool.tile([P, D], BF16, name=f"sqscr{r}", tag="sqscr")
            ssq_c = stats.tile([P, 1], F32, name=f"ssqc{r}", tag="ssqc")
            ssq_u = stats.tile([P, 1], F32, name=f"ssqu{r}", tag="ssqu")
            nc.vector.tensor_mul(sq_scr, csum, csum)
            nc.vector.tensor_reduce(ssq_c, sq_scr, axis=mybir.AxisListType.X,
                                    op=mybir.AluOpType.add)
            nc.vector.tensor_mul(sq_scr, u, u)
            nc.vector.tensor_reduce(ssq_u, sq_scr, axis=mybir.AxisListType.X,
                                    op=mybir.AluOpType.add)
            lc = stats.tile([P, 1], F32, name=f"lc{r}", tag="lc")
            lu = stats.tile([P, 1], F32, name=f"lu{r}", tag="lu")
            nc.scalar.activation(lc, ssq_c, AF.Ln)
            nc.scalar.activation(lu, ssq_u, AF.Ln)
            rc = stats.tile([P, 1], F32, name=f"rc{r}", tag="rc")
            ru = stats.tile([P, 1], F32, name=f"ru{r}", tag="ru")
            nc.scalar.activation(rc, lc, AF.Exp, scale=-0.5)
            nc.scalar.activation(ru, lu, AF.Exp, scale=-0.5)

            # normalize (f32 out so the PSUM->SBUF copies can ride ScalarE)
            chat = cpool.tile([P, D], F32, name=f"chat{r}", tag="chat")
            uhat = cpool.tile([P, D], F32, name=f"uhat{r}", tag="uhat")
            nc.vector.tensor_scalar_mul(chat, csum, rc)
            nc.vector.tensor_scalar_mul(uhat, u, ru)

            # diagonal cos (local)
            dg_scr = cpool.tile([P, D], F32, name=f"dgscr{r}", tag="dgscr")
            nc.vector.tensor_mul(dg_scr, chat, uhat)
            nc.vector.tensor_reduce(diag_all[:, r:r + 1], dg_scr,
                                    axis=mybir.AxisListType.X,
                                    op=mybir.AluOpType.add)

            # transposes on PE (f32 in -> f32 psum), cast to bf16 on ScalarE
            cT = cpool.tile([P, NT, P], FP8, name=f"cT{r}", tag="cT")
            for t in range(NT):
                pt = tpsum.tile([P, P], F32, name=f"ptc{r}_{t}", tag="ptc")
                nc.tensor.transpose(pt, chat[:, t * P:(t + 1) * P], ident)
                nc.scalar.copy(cT[:, t, :], pt)
                pu = tpsum.tile([P, P], F32, name=f"ptu{r}_{t}", tag="ptu")
                nc.tensor.transpose(pu, uhat[:, t * P:(t + 1) * P], ident)
                nc.scalar.copy(uT[:, t, r * P:(r + 1) * P], pu)

            # bounce write + allgather (bf16)
            bounce = dram.tile([D, P], FP8, name=f"bounce{r}")
            nc.sync.dma_start(out=bounce.rearrange("(t p) i -> p t i", p=P),
                              in_=cT)
            g = dram.tile([N_CORES * D, P], FP8, name=f"gath{r}",
                          addr_space="Shared")
            nc.gpsimd.collective_compute(
                "AllGather", mybir.AluOpType.bypass,
                replica_groups=[list(range(N_CORES))],
                ins=[bounce.opt()], outs=[g.opt()],
            )
            gath.append(g)

        # ---------- phase C: gathered reads + matmuls + epilogue ----------
        for r in range(NCHUNK):
            g_sb = gpool.tile([P, N_CORES * NT, P], FP8, name=f"gsb{r}",
                              tag="gsb")
            nc.sync.dma_start(
                out=g_sb,
                in_=gath[r].rearrange("(c t p) i -> p (c t) i", t=NT, p=P))
            for q in range(NCHUNK):
                ps = mpsum.tile([P, 2, N_CORES // 2 * P], F32,
                                name=f"ps{r}_{q}", tag="ps")
                for h in range(2):
                    for t in range(NT):
                        # rhs: ranks c in [4h, 4h+4), d-tile t -> N=512
                        rhs = g_sb[:, 16 * h + t:16 * h + t + 13:NT, :]
                        nc.tensor.matmul(
                            ps[:, h, :],
                            uT[:, t, q * P:(q + 1) * P],
                            rhs,
                            start=(t == 0), stop=(t == NT - 1),
                        )
                # epilogue: y = max(cos, eps); s += sum_k exp(w*y)
                y = epool.tile([P, 2 * (N_CORES // 2) * P], BF16,
                               name=f"y{r}_{q}", tag="y")
                nc.vector.tensor_scalar_max(y, ps.rearrange("p a b -> p (a b)"),
                                            EPS)
                e_scr = epool.tile([P, 2 * (N_CORES // 2) * P], BF16,
                                   name=f"escr{r}_{q}", tag="escr")
                s_part = stats.tile([P, 1], F32, name=f"sp{r}_{q}", tag="sp")
                nc.scalar.activation(e_scr, y, AF.Exp, scale=w_val,
                                     accum_out=s_part)
                nc.vector.tensor_add(s_acc[:, q:q + 1], s_acc[:, q:q + 1],
                                     s_part)

        # ---------- finals: rows = ln(s) - w*max(diag, eps) ----------
        dgc = singles.tile([P, NCHUNK], F32)
        nc.vector.tensor_scalar_max(dgc, diag_all, EPS)
        lse = singles.tile([P, NCHUNK], F32)
        nc.scalar.activation(lse, s_acc, AF.Ln)
        nc.vector.tensor_scalar(rows, dgc, -w_val, None, mybir.AluOpType.mult)
        nc.vector.tensor_add(rows, rows, lse)
        nc.sync.dma_start(out=out.rearrange("(q p) -> p q", p=P), in_=rows)


_CACHE = {}


def kernel(dvecs, w, b):
    w_val = float(np.asarray(w))
    key = w_val
    if key not in _CACHE:
        _CACHE[key] = build_program(w_val)
    nc = _CACHE[key]
    dvecs = np.ascontiguousarray(np.asarray(dvecs, dtype=np.float32))
    in_maps = [
        {"dvecs": dvecs[c * LOCAL:(c + 1) * LOCAL]} for c in range(N_CORES)
    ]
    res = run_bass_kernel_spmd(nc, in_maps, core_ids=list(range(N_CORES)))
    total = 0.0
    for c in range(N_CORES):
        total += float(np.asarray(res.results[c]["out"], dtype=np.float64).sum())
    return np.float32(total / N)
